# revision 14
# baseline (speedup 1.0000x reference)
"""DiffLinearAttention Trainium2 kernel.

Full inputs in, full outputs out. Sharding: 16 heads / 8 cores = 2 heads per
core (head-parallel SPMD, no collectives). Each core computes its two heads
with a chunked causal linear-attention algorithm (chunk C=128):

  per head h, per stream s in {1,2}:
    qs = feature map of Q (transposed layout [d, l]), ks likewise
    kn = feature map of K (normal layout [m, d])
    KV_c = sum_{m in chunk c} kn_m * [v_m | 1]      (prefix accumulated in PSUM)
    U[l]  = qs_l @ KV_{<chunk(l)}  +  sum_{m<=l, same chunk} (qs_l . ks_m) [v_m|1]
    out   = U1[:, :64]/(U1[:,64]+eps) - lambda * U2[:, :64]/(U2[:,64]+eps)

Partition packing: head 0 occupies SBUF partitions 0-63, head 1 partitions
64-127 for all [d, *]-shaped tensors, so elementwise feature ops process both
heads in single full-width instructions.
"""

import numpy as np

import concourse.bass as bass
from concourse import bacc
import concourse.mybir as mybir
from concourse.tile import TileContext
from concourse.bass_utils import run_bass_kernel_spmd
from concourse.masks import make_identity, make_upper_triangular

F32 = mybir.dt.float32
AF = mybir.ActivationFunctionType
OP = mybir.AluOpType

H, L, D = 16, 2048, 64
N_CORES = 8
HPC = H // N_CORES  # heads per core
C = 128             # chunk size
NCH = L // C        # 16 chunks
EPS = 1e-6
LAMBDA_INIT = 0.2
NSPLIT = 4          # n-chunks of 512 for feature matmuls


def _hrows(h):
    return slice(h * 64, (h + 1) * 64)


def _ccols(c):
    return slice(c * C, (c + 1) * C)


def build_program() -> bass.Bass:
    nc = bacc.Bacc()
    q_d = nc.declare_dram_parameter("q", [HPC, L, D], F32, isOutput=False)
    k_d = nc.declare_dram_parameter("k", [HPC, L, D], F32, isOutput=False)
    v_d = nc.declare_dram_parameter("v", [HPC, L, D], F32, isOutput=False)
    wq1_d = nc.declare_dram_parameter("wq1", [HPC, D, D], F32, isOutput=False)
    wq2_d = nc.declare_dram_parameter("wq2", [HPC, D, D], F32, isOutput=False)
    wk1_d = nc.declare_dram_parameter("wk1", [HPC, D, D], F32, isOutput=False)
    wk2_d = nc.declare_dram_parameter("wk2", [HPC, D, D], F32, isOutput=False)
    nlam_d = nc.declare_dram_parameter("neg_lam", [128, 1], F32, isOutput=False)
    out_d = nc.declare_dram_parameter("out", [HPC, L, D], F32, isOutput=True)

    with TileContext(nc) as tc:
        _body(nc, tc, q_d, k_d, v_d, wq1_d, wq2_d, wk1_d, wk2_d, nlam_d, out_d)
    nc.finalize()
    return nc


def _body(nc, tc, q_d, k_d, v_d, wq1_d, wq2_d, wk1_d, wk2_d, nlam_d, out_d):
    from contextlib import ExitStack

    ctx = ExitStack()
    const = ctx.enter_context(tc.tile_pool(name="const", bufs=1))
    io = ctx.enter_context(tc.tile_pool(name="io", bufs=1))
    feat = ctx.enter_context(tc.tile_pool(name="feat", bufs=1))
    work = ctx.enter_context(tc.tile_pool(name="work", bufs=4))

    # ---- constants -------------------------------------------------------
    ident = const.tile([128, 128], F32)
    make_identity(nc, ident)
    maskut = const.tile([128, 128], F32)
    make_upper_triangular(nc, maskut, val=1.0, diag=True)

    # weights, head-packed on partitions: rows 0-63 head0, 64-127 head1
    wq1_sb = const.tile([128, D], F32)
    wq2_sb = const.tile([128, D], F32)
    wk1_sb = const.tile([128, D], F32)
    wk2_sb = const.tile([128, D], F32)
    wkstack = const.tile([128, 2 * D], F32)  # [wk1 | wk2] along free dim
    for h in range(HPC):
        nc.sync.dma_start(out=wq1_sb[_hrows(h), :], in_=wq1_d[h])
        nc.sync.dma_start(out=wq2_sb[_hrows(h), :], in_=wq2_d[h])
        nc.sync.dma_start(out=wk1_sb[_hrows(h), :], in_=wk1_d[h])
        nc.sync.dma_start(out=wk2_sb[_hrows(h), :], in_=wk2_d[h])
        nc.sync.dma_start(out=wkstack[_hrows(h), 0:D], in_=wk1_d[h])
        nc.sync.dma_start(out=wkstack[_hrows(h), D : 2 * D], in_=wk2_d[h])
    nlam = const.tile([128, 1], F32)
    nc.sync.dma_start(out=nlam, in_=nlam_d[:, :])

    # ---- input loads -----------------------------------------------------
    q_in, k_in, v_sb = [], [], []
    for h in range(HPC):
        qi = io.tile([128, NCH, D], F32, name=f"q_in{h}", tag=f"q_in{h}")
        ki = io.tile([128, NCH, D], F32, name=f"k_in{h}", tag=f"k_in{h}")
        vi = io.tile([128, NCH, D + 1], F32, name=f"v_sb{h}", tag=f"v_sb{h}")
        nc.sync.dma_start(out=qi, in_=q_d[h].rearrange("(c p) d -> p c d", p=128))
        nc.sync.dma_start(out=ki, in_=k_d[h].rearrange("(c p) d -> p c d", p=128))
        nc.sync.dma_start(
            out=vi[:, :, 0:D], in_=v_d[h].rearrange("(c p) d -> p c d", p=128)
        )
        nc.gpsimd.memset(vi[:, :, D : D + 1], 1.0)
        q_in.append(qi)
        k_in.append(ki)
        v_sb.append(vi)

    # ---- persistent SBUF feature tensors (head-packed rows) --------------
    qT = feat.tile([128, L], F32)   # [d, l] both heads
    kT = feat.tile([128, L], F32)
    qs1 = feat.tile([128, L], F32)  # stream-1 Q features, transposed layout
    qs2 = feat.tile([128, L], F32)
    ks1 = feat.tile([128, L], F32)
    ks2 = feat.tile([128, L], F32)
    kn1 = [feat.tile([128, NCH, D], F32, name=f"kn1_{h}") for h in range(HPC)]
    kn2 = [feat.tile([128, NCH, D], F32, name=f"kn2_{h}") for h in range(HPC)]
    kvs = [feat.tile([128, NCH, D + 1], F32, name=f"kvs{s}") for s in range(2)]

    with (
        tc.tile_pool(name="ps_a", bufs=4, space="PSUM") as ps_a,
        tc.tile_pool(name="ps_kv", bufs=4, space="PSUM") as ps_kv,
    ):
        # ---- transposes: q/k [l, d] chunks -> qT/kT [d, l] ---------------
        for src, dst in ((q_in, qT), (k_in, kT)):
            for c in range(NCH):
                tp = ps_a.tile([128, 128], F32, name="tp", tag="ps_a")
                nc.tensor.matmul(
                    tp[0:64, :], src[0][:, c, :], ident, start=True, stop=True
                )
                nc.tensor.matmul(
                    tp[64:128, :], src[1][:, c, :], ident, start=True, stop=True
                )
                if c % 2 == 0:
                    nc.vector.tensor_copy(dst[:, _ccols(c)], tp)
                else:
                    nc.scalar.copy(dst[:, _ccols(c)], tp)

        # ---- feature maps, transposed layout (both heads at once) --------
        for w1_sb, w2_sb, src, f1, f2 in (
            (wq1_sb, wq2_sb, qT, qs1, qs2),
            (wk1_sb, wk2_sb, kT, ks1, ks2),
        ):
            for n in range(NSPLIT):
                ncols = slice(n * (L // NSPLIT), (n + 1) * (L // NSPLIT))
                z1 = ps_a.tile([128, L // NSPLIT], F32, name="z1", tag="ps_a")
                z2 = ps_a.tile([128, L // NSPLIT], F32, name="z2", tag="ps_a")
                nc.tensor.matmul(
                    z1[0:64, :], w1_sb[0:64, :], src[0:64, ncols],
                    start=True, stop=True,
                )
                nc.tensor.matmul(
                    z1[64:128, :], w1_sb[64:128, :], src[64:128, ncols],
                    start=True, stop=True,
                )
                nc.tensor.matmul(
                    z2[0:64, :], w2_sb[0:64, :], src[0:64, ncols],
                    start=True, stop=True,
                )
                nc.tensor.matmul(
                    z2[64:128, :], w2_sb[64:128, :], src[64:128, ncols],
                    start=True, stop=True,
                )
                # f1 = relu(z1)
                nc.scalar.activation(f1[:, ncols], z1, AF.Relu)
                # f2 = sigmoid(relu(z2)) * f1
                sg = work.tile([128, L // NSPLIT], F32, name="sg", tag="sg")
                nc.scalar.activation(sg, z2, AF.Relu)
                nc.scalar.activation(sg, sg, AF.Sigmoid)
                nc.vector.tensor_mul(f2[:, ncols], sg, f1[:, ncols])

        # ---- K features, normal layout [m, d] ----------------------------
        for h in range(HPC):
            for cq in range(NCH // 4):
                zn = ps_a.tile([128, 512], F32, name="zn", tag="ps_a")
                for j in range(4):
                    c = 4 * cq + j
                    nc.tensor.matmul(
                        zn[:, j * 128 : (j + 1) * 128],
                        kT[_hrows(h), _ccols(c)],
                        wkstack[_hrows(h), :],
                        start=True, stop=True,
                    )
                znr = zn.rearrange("p (j e) -> p j e", j=4)
                cs = slice(4 * cq, 4 * cq + 4)
                nc.scalar.activation(kn1[h][:, cs, :], znr[:, :, 0:D], AF.Relu)
                sgn = work.tile([128, 4, D], F32, name="sgn", tag="sgn")
                nc.scalar.activation(sgn, znr[:, :, D : 2 * D], AF.Relu)
                nc.scalar.activation(sgn, sgn, AF.Sigmoid)
                nc.vector.tensor_mul(kn2[h][:, cs, :], sgn, kn1[h][:, cs, :])

        # ---- KV prefix chains -------------------------------------------
        # per-chunk outer products in PSUM (independent matmuls), prefix-
        # summed into kvs in SBUF by vector adds (both heads per op).
        for s in range(2):
            nc.gpsimd.memset(kvs[s][:, 0, :], 0.0)
        for c in range(NCH - 1):
            for s, kn_s in ((0, kn1), (1, kn2)):
                kvp = ps_kv.tile([128, D + 1], F32, name="kvp", tag="kvp")
                for h in range(HPC):
                    nc.tensor.matmul(
                        kvp[_hrows(h), :], kn_s[h][:, c, :], v_sb[h][:, c, :],
                        start=True, stop=True,
                    )
                nc.vector.tensor_add(kvs[s][:, c + 1, :], kvp, kvs[s][:, c, :])

    # ---- main attention loop --------------------------------------------
    with (
        tc.tile_pool(name="ps_s", bufs=2, space="PSUM") as ps_s,
        tc.tile_pool(name="ps_u", bufs=2, space="PSUM") as ps_u,
    ):
        u_done = [[None] * 2 for _ in range(HPC)]
        for c in range(NCH):
            for h in range(HPC):
                hr = _hrows(h)
                for s, qs_s, ks_s in ((0, qs1, ks1), (1, qs2, ks2)):
                    # intra-chunk scores S^T[m, l] for this chunk
                    stp = ps_s.tile([128, 128], F32, name="stp", tag="st")
                    nc.tensor.matmul(
                        stp, ks_s[hr, _ccols(c)], qs_s[hr, _ccols(c)],
                        start=True, stop=True,
                    )
                    stm = work.tile([128, 128], F32, name="stm", tag="stm")
                    nc.vector.tensor_mul(stm, stp, maskut)  # causal mask l>=m

                    u_ps = ps_u.tile([128, D + 1], F32, name="u_ps", tag=f"u{s}")
                    if c > 0:
                        mi = nc.tensor.matmul(
                            u_ps, qs_s[hr, _ccols(c)], kvs[s][hr, c, :],
                            start=True, stop=False,
                        )
                        nc.tensor.matmul(
                            u_ps, stm, v_sb[h][:, c, :], start=False, stop=True
                        )
                    else:
                        nc.tensor.matmul(
                            u_ps, stm, v_sb[h][:, c, :], start=True, stop=True
                        )
                    u_done[h][s] = u_ps

                # epilogue for (h, c): out = U1*inv1 + U2*inv2*(-lam)
                u1, u2 = u_done[h]
                o1 = work.tile([128, D], F32, name="o1", tag="o1")
                t2 = work.tile([128, D], F32, name="t2", tag="t2")
                rp1 = work.tile([128, 1], F32, name="rp1", tag="rp")
                rp2 = work.tile([128, 1], F32, name="rp2", tag="rp")
                nc.vector.tensor_scalar_add(rp1, u1[:, D : D + 1], EPS)
                nc.vector.tensor_scalar_add(rp2, u2[:, D : D + 1], EPS)
                iv1 = work.tile([128, 1], F32, name="iv1", tag="iv")
                iv2 = work.tile([128, 1], F32, name="iv2", tag="iv")
                nc.vector.reciprocal(iv1, rp1)
                nc.vector.reciprocal(iv2, rp2)
                nc.scalar.activation(o1, u1[:, 0:D], AF.Copy, scale=iv1)
                nc.vector.tensor_scalar(
                    t2, u2[:, 0:D], iv2, nlam[:, 0:1], OP.mult, OP.mult
                )
                ob = work.tile([128, D], F32, name="ob", tag="ob")
                nc.vector.tensor_add(ob, o1, t2)
                nc.sync.dma_start(
                    out=out_d[h].rearrange("(c p) d -> p c d", p=128)[:, c, :],
                    in_=ob,
                )

    ctx.close()


_PROGRAM_CACHE = {}


def _get_program():
    if "nc" not in _PROGRAM_CACHE:
        _PROGRAM_CACHE["nc"] = build_program()
    return _PROGRAM_CACHE["nc"]


def kernel(
    query_states, key_states, value_states,
    W_q1, W_k1, W_q2, W_k2,
    lambda_q1, lambda_k1, lambda_q2, lambda_k2,
):
    q = np.asarray(query_states, dtype=np.float32)[0]  # [H, L, D]
    k = np.asarray(key_states, dtype=np.float32)[0]
    v = np.asarray(value_states, dtype=np.float32)[0]

    lam1 = np.sum(np.asarray(lambda_q1, np.float32) * np.asarray(lambda_k1, np.float32),
                  dtype=np.float32)
    lam2 = np.sum(np.asarray(lambda_q2, np.float32) * np.asarray(lambda_k2, np.float32),
                  dtype=np.float32)
    lam = np.float32(np.tanh(max(np.float32(lam1 - lam2 + np.float32(LAMBDA_INIT)),
                                 np.float32(0.0))))
    neg_lam = np.full((128, 1), -lam, np.float32)

    nc = _get_program()
    in_maps = []
    for i in range(N_CORES):
        hs = slice(HPC * i, HPC * (i + 1))
        in_maps.append({
            "q": q[hs], "k": k[hs], "v": v[hs],
            "wq1": np.asarray(W_q1, np.float32)[hs],
            "wq2": np.asarray(W_q2, np.float32)[hs],
            "wk1": np.asarray(W_k1, np.float32)[hs],
            "wk2": np.asarray(W_k2, np.float32)[hs],
            "neg_lam": neg_lam,
        })
    res = run_bass_kernel_spmd(nc, in_maps, list(range(N_CORES)))
    out = np.concatenate([res.results[i]["out"] for i in range(N_CORES)], axis=0)
    return out[None]  # [1, H, L, D]
